# revision 1
# baseline (speedup 1.0000x reference)
"""Trainium2 Bass kernel for nn_CoreAttention (causal attention).

Problem (hardcoded): Q/K/V [SQ=2048, B=2, H=16, D=64] fp32, causal mask,
softmax(QK^T/8) @ V, output [2048, 2, 1024].

Sharding: batch*heads (32) split 4 heads per core across 8 cores.

Per-core device layout (host prepares these in the shard step):
  qt  [256, 2048] f32r : Q^T d-major; row = pair*128 + head_local*64 + d
  kt  [256, 2048] f32r : K^T same layout
  v   [4, 2048, 64]    : V natural per head
  out [4, 64, 2048] f32: context^T per head (normalized); host transposes back

Algorithm per head-pair (2 heads packed on 128 SBUF partitions):
  For each q-block j (512 wide), accumulate over k-blocks i (128 wide,
  causally trimmed): S^T = K_blk^T^T.T @ Q^T via PE row-tiled pair
  (head A rows 0-63, head B rows 64-127), additive causal mask on the
  diagonal 128x128 sub-block, exp on ScalarE (scale=1/8), then
  ctx^T[65, 512] += V'_blk.T @ P^T on PE where V' has a ones column
  (row 64 of ctx^T = softmax denominator). Epilogue: reciprocal +
  partition-broadcast + multiply, DMA out.
"""

import os
import sys

sys.path.insert(0, "/opt/trn_rl_repo")

import numpy as np

from contextlib import ExitStack

import concourse.bass as bass
import concourse.mybir as mybir
import concourse.tile as tile
from concourse import bacc

SQ, B, H, D = 2048, 2, 16, 64
NCORES = 8
HPC = 4  # heads per core
NPAIR = 2  # head pairs per core
KB = 128  # k block
QB = 512  # q block
NKB = SQ // KB  # 16
NQB = SQ // QB  # 4
NORM = 8.0  # sqrt(D) * layer_number
MASK_FILL = -30000.0

F32 = mybir.dt.float32
F32R = mybir.dt.float32r


def build_attention(nc, tc, ctx_stack, reps=1):
    qt = nc.dram_tensor("qt", [NPAIR * 128, SQ], F32R, kind="ExternalInput").ap()
    kt = nc.dram_tensor("kt", [NPAIR * 128, SQ], F32R, kind="ExternalInput").ap()
    # v carries a host-prepared ones column at d=D (softmax denominator trick).
    v = nc.dram_tensor("v", [HPC, SQ, D + 1], F32R, kind="ExternalInput").ap()
    out = nc.dram_tensor("out", [HPC, D, SQ], F32, kind="ExternalOutput").ap()

    ec = ctx_stack.enter_context
    consts = ec(tc.tile_pool(name="consts", bufs=1))
    inp = ec(tc.tile_pool(name="inp", bufs=1))
    pp = ec(tc.tile_pool(name="pp", bufs=4))
    ep = ec(tc.tile_pool(name="ep", bufs=3))
    psum_s = ec(tc.tile_pool(name="psum_s", bufs=3, space="PSUM"))
    psum_c = ec(tc.tile_pool(name="psum_c", bufs=1, space="PSUM"))

    # Additive causal mask for the diagonal 128x128 sub-block in S^T layout
    # (partition = k, free = q): keep where q >= k else MASK_FILL.
    mask_sb = consts.tile([128, 128], F32)
    nc.gpsimd.memset(mask_sb, 0.0)
    nc.gpsimd.affine_select(
        out=mask_sb,
        in_=mask_sb,
        compare_op=mybir.AluOpType.is_ge,
        fill=MASK_FILL,
        base=0,
        pattern=[[1, 128]],  # iota over free dim: +q
        channel_multiplier=-1,  # -k per partition
    )

    # Resident inputs.
    qt_sb = inp.tile([128, NPAIR, SQ], F32R)
    kt_sb = inp.tile([128, NPAIR, SQ], F32R)
    vp_sb = inp.tile([128, HPC, NKB, D + 1], F32R)

    # Chunked input loads, ordered by first use (j runs descending, k
    # ascending): kt chunks ascending, qt chunks descending, vp ascending.
    qt_r = qt.rearrange("(pr p) q -> p pr q", p=128)
    kt_r = kt.rearrange("(pr p) q -> p pr q", p=128)
    v_r = [v[g].rearrange("(n p) d -> p n d", p=128) for g in range(HPC)]
    for c in range(NQB):
        ksl = slice(c * QB, (c + 1) * QB)
        qsl = slice((NQB - 1 - c) * QB, (NQB - c) * QB)
        for pr in range(NPAIR):
            nc.sync.dma_start(out=kt_sb[:, pr, ksl], in_=kt_r[:, pr, ksl])
        for pr in range(NPAIR):
            nc.sync.dma_start(out=qt_sb[:, pr, qsl], in_=qt_r[:, pr, qsl])
        bl = slice(4 * c, 4 * c + 4)
        for g in range(HPC):
            nc.sync.dma_start(out=vp_sb[:, g, bl, :], in_=v_r[g][:, bl, :])

    # j descending (longest i-loops first, so the kernel tail is short).
    for _rep in range(reps):
      for pr in range(NPAIR):
        for j in range(NQB - 1, -1, -1):
            n_i = 4 * j + 4  # causal: k blocks 0 .. 4j+3
            ctx_A = psum_c.tile([128, QB], F32, tag="ctxA", name="ctxA")
            ctx_B = psum_c.tile([128, QB], F32, tag="ctxB", name="ctxB")
            for i in range(n_i):
                t = i - 4 * j
                qs = max(0, 128 * t)  # q start within the 512 block
                s_ps = psum_s.tile([128, 2 * QB], F32, tag="s")
                # BMM1: S^T[k, q] for both heads, row-tiled on the PE.
                nc.tensor.matmul(
                    s_ps[:, qs:QB],
                    lhsT=kt_sb[0:64, pr, i * KB : (i + 1) * KB],
                    rhs=qt_sb[0:64, pr, j * QB + qs : (j + 1) * QB],
                    start=True,
                    stop=True,
                    tile_position=(0, 0),
                )
                nc.tensor.matmul(
                    s_ps[:, QB + qs : 2 * QB],
                    lhsT=kt_sb[64:128, pr, i * KB : (i + 1) * KB],
                    rhs=qt_sb[64:128, pr, j * QB + qs : (j + 1) * QB],
                    start=True,
                    stop=True,
                    tile_position=(64, 0),
                )
                p_sb = pp.tile([128, 2 * QB], F32R, tag="p")
                s3 = s_ps.rearrange("p (h q) -> p h q", h=2)
                p3 = p_sb.rearrange("p (h q) -> p h q", h=2)
                if t >= 0:
                    # Diagonal sub-block: additive causal mask on both heads.
                    nc.vector.tensor_add(
                        s3[:, :, qs : qs + 128],
                        s3[:, :, qs : qs + 128],
                        mask_sb.unsqueeze(1).broadcast_to((128, 2, 128)),
                    )
                nc.scalar.activation(
                    p3[:, :, qs:QB],
                    s3[:, :, qs:QB],
                    mybir.ActivationFunctionType.Exp,
                    scale=1.0 / NORM,
                )
                # BMM2: ctx^T[0:64] += V.T @ P^T ; row 64 accumulates sums.
                nc.tensor.matmul(
                    ctx_A[0 : D + 1, qs:QB],
                    lhsT=vp_sb[:, 2 * pr, i, :],
                    rhs=p_sb[:, qs:QB],
                    start=(i == 0),
                    stop=(i == n_i - 1),
                )
                nc.tensor.matmul(
                    ctx_B[0 : D + 1, qs:QB],
                    lhsT=vp_sb[:, 2 * pr + 1, i, :],
                    rhs=p_sb[:, QB + qs : 2 * QB],
                    start=(i == 0),
                    stop=(i == n_i - 1),
                )
            # Epilogue: normalize and store ctx^T for both heads.
            for hl, ctx in ((0, ctx_A), (1, ctx_B)):
                g = 2 * pr + hl
                recip = ep.tile([1, QB], F32, tag="recip")
                nc.vector.reciprocal(recip, ctx[D : D + 1, :])
                bcast = ep.tile([64, QB], F32, tag="bcast")
                nc.gpsimd.partition_broadcast(bcast, recip)
                ctxn = ep.tile([64, QB], F32, tag="ctxn")
                nc.vector.tensor_mul(ctxn, ctx[0:D, :], bcast)
                nc.sync.dma_start(
                    out=out[g, :, j * QB : (j + 1) * QB], in_=ctxn
                )


def _build_nc():
    nc = bacc.Bacc(
        "TRN2", target_bir_lowering=False, debug=False, num_devices=NCORES
    )
    with tile.TileContext(nc) as tc, ExitStack() as ctx_stack:
        build_attention(nc, tc, ctx_stack)
    nc.compile()
    return nc


_NC_CACHE = {}


def get_nc():
    if "nc" not in _NC_CACHE:
        _NC_CACHE["nc"] = _build_nc()
    return _NC_CACHE["nc"]


def shard_inputs(query_layer, key_layer, value_layer):
    """Full [SQ, B, H, D] fp32 inputs -> list of 8 per-core input dicts."""
    q = np.asarray(query_layer, dtype=np.float32)
    k = np.asarray(key_layer, dtype=np.float32)
    v = np.asarray(value_layer, dtype=np.float32)
    # [SQ, B, H, D] -> [B*H, D, SQ] (d-major) for Q/K; [B*H, SQ, D] for V.
    qt = np.ascontiguousarray(q.transpose(1, 2, 3, 0).reshape(B * H, D, SQ))
    kt = np.ascontiguousarray(k.transpose(1, 2, 3, 0).reshape(B * H, D, SQ))
    vn = v.transpose(1, 2, 0, 3).reshape(B * H, SQ, D)
    vn = np.ascontiguousarray(
        np.concatenate([vn, np.ones((B * H, SQ, 1), np.float32)], axis=2)
    )
    in_maps = []
    for c in range(NCORES):
        sl = slice(HPC * c, HPC * (c + 1))
        in_maps.append(
            {
                "qt": np.ascontiguousarray(qt[sl].reshape(HPC * D, SQ)),
                "kt": np.ascontiguousarray(kt[sl].reshape(HPC * D, SQ)),
                "v": np.ascontiguousarray(vn[sl]),
            }
        )
    return in_maps


def gather_outputs(results):
    """8 per-core {'out': [4, 64, 2048]} -> full [SQ, B, H*D] fp32."""
    ctx_t = np.stack([results[c]["out"] for c in range(NCORES)])  # [8,4,64,SQ]
    ctx_t = ctx_t.reshape(B * H, D, SQ)
    full = ctx_t.transpose(2, 0, 1).reshape(SQ, B, H * D)
    return np.ascontiguousarray(full.astype(np.float32))


def run_on_device(in_maps, trace=False):
    from concourse.bass_utils import run_bass_kernel_spmd

    nc = get_nc()
    res = run_bass_kernel_spmd(
        nc, in_maps, core_ids=list(range(NCORES)), trace=trace
    )
    return res


def kernel(query_layer, key_layer, value_layer, attention_mask=None):
    in_maps = shard_inputs(query_layer, key_layer, value_layer)
    res = run_on_device(in_maps, trace=False)
    return gather_outputs(res.results)



# revision 2
# speedup vs baseline: 2.0988x; 2.0988x over previous
"""Trainium2 Bass kernel for nn_CoreAttention (causal attention).

Problem (hardcoded): Q/K/V [SQ=2048, B=2, H=16, D=64] fp32, causal mask,
softmax(QK^T/8) @ V, output [2048, 2, 1024].

Sharding: batch*heads (32) split 4 heads per core across 8 cores.

Per-core device layout (host prepares these in the shard step):
  qt  [256, 2048] bf16 : Q^T d-major; row = pair*128 + head_local*64 + d
  kt  [256, 2048] bf16 : K^T same layout
  v   [4, 2048, 65]    : V natural per head + ones column (denominator)
  out [4, 65, 2048] f32: rows 0-63 unnormalized context^T, row 64 the
                         softmax denominator; host divides + transposes.

Algorithm per head-pair (2 heads packed on 128 SBUF partitions):
  For each q-block j (512 wide), software-pipelined over k-blocks i
  (128 wide, causally trimmed):
    BMM1  S^T[k, q] for both heads via PE row-tiled pair -> PSUM [128,2,512]
    exp   ScalarE activation (scale=1/8) PSUM -> bf16 P in SBUF
    mask  diagonal 128x128 sub-block zeroed on GpSimd (affine_select)
          after exp (keeps mask off the BMM1->exp critical path)
    BMM2  ctx^T[65, q] += V'.T @ P^T per head (V' has ones column; row 64
          accumulates the denominator), issued one i behind BMM1 so the
          PE never waits on the ACT engine.
  Epilogue: DVE copy ctx PSUM->SBUF, DMA out (no on-device normalize).
"""

import sys

sys.path.insert(0, "/opt/trn_rl_repo")

import numpy as np

from contextlib import ExitStack

import concourse.bass as bass
import concourse.mybir as mybir
import concourse.tile as tile
from concourse import bacc

SQ, B, H, D = 2048, 2, 16, 64
NCORES = 8
HPC = 4  # heads per core
NPAIR = 2  # head pairs per core
KB = 128  # k block
QB = 512  # q block
NKB = SQ // KB  # 16
NQB = SQ // QB  # 4
NORM = 8.0  # sqrt(D) * layer_number

F32 = mybir.dt.float32
BF16 = mybir.dt.bfloat16


def build_attention(nc, tc, ctx_stack):
    qt = nc.dram_tensor("qt", [NPAIR * 128, SQ], BF16, kind="ExternalInput").ap()
    kt = nc.dram_tensor("kt", [NPAIR * 128, SQ], BF16, kind="ExternalInput").ap()
    # v carries a host-prepared ones column at d=D (softmax denominator trick).
    v = nc.dram_tensor("v", [HPC, SQ, D + 1], BF16, kind="ExternalInput").ap()
    out = nc.dram_tensor("out", [HPC, D + 1, SQ], F32, kind="ExternalOutput").ap()

    ec = ctx_stack.enter_context
    inp = ec(tc.tile_pool(name="inp", bufs=1))
    pp = ec(tc.tile_pool(name="pp", bufs=3))
    ep = ec(tc.tile_pool(name="ep", bufs=4))
    psum_s = ec(tc.tile_pool(name="psum_s", bufs=2, space="PSUM"))
    psum_c = ec(tc.tile_pool(name="psum_c", bufs=2, space="PSUM"))

    # Resident inputs.
    qt_sb = inp.tile([128, NPAIR, SQ], BF16)
    kt_sb = inp.tile([128, NPAIR, SQ], BF16)
    vp_sb = inp.tile([128, HPC, NKB, D + 1], BF16)

    # Chunked input loads, ordered by first use (j runs descending, k
    # ascending): kt chunks ascending, qt chunks descending, vp ascending.
    qt_r = qt.rearrange("(pr p) q -> p pr q", p=128)
    kt_r = kt.rearrange("(pr p) q -> p pr q", p=128)
    v_r = [v[g].rearrange("(n p) d -> p n d", p=128) for g in range(HPC)]
    for c in range(NQB):
        ksl = slice(c * QB, (c + 1) * QB)
        qsl = slice((NQB - 1 - c) * QB, (NQB - c) * QB)
        for pr in range(NPAIR):
            nc.sync.dma_start(out=kt_sb[:, pr, ksl], in_=kt_r[:, pr, ksl])
        for pr in range(NPAIR):
            nc.sync.dma_start(out=qt_sb[:, pr, qsl], in_=qt_r[:, pr, qsl])
        bl = slice(4 * c, 4 * c + 4)
        for g in range(HPC):
            nc.sync.dma_start(out=vp_sb[:, g, bl, :], in_=v_r[g][:, bl, :])

    def mm1_pair(pr, j, i, s_ps):
        """BMM1: S^T[k, q] for both heads, row-tiled on the PE."""
        t = i - 4 * j
        qs = max(0, 128 * t)
        nc.tensor.matmul(
            s_ps[:, 0, qs:QB],
            lhsT=kt_sb[0:64, pr, i * KB : (i + 1) * KB],
            rhs=qt_sb[0:64, pr, j * QB + qs : (j + 1) * QB],
            start=True,
            stop=True,
            tile_position=(0, 0),
        )
        nc.tensor.matmul(
            s_ps[:, 1, qs:QB],
            lhsT=kt_sb[64:128, pr, i * KB : (i + 1) * KB],
            rhs=qt_sb[64:128, pr, j * QB + qs : (j + 1) * QB],
            start=True,
            stop=True,
            tile_position=(64, 0),
        )

    def mm2_pair(pr, j, i, n_i, p_sb, qs, ctx_A, ctx_B):
        """BMM2: ctx^T[0:64] += V.T @ P^T ; row 64 accumulates sums."""
        nc.tensor.matmul(
            ctx_A[:, qs:QB],
            lhsT=vp_sb[:, 2 * pr, i, :],
            rhs=p_sb[:, 0, qs:QB],
            start=(i == 0),
            stop=(i == n_i - 1),
        )
        nc.tensor.matmul(
            ctx_B[:, qs:QB],
            lhsT=vp_sb[:, 2 * pr + 1, i, :],
            rhs=p_sb[:, 1, qs:QB],
            start=(i == 0),
            stop=(i == n_i - 1),
        )

    # j descending (longest i-loops first, so the kernel tail is short).
    for pr in range(NPAIR):
        for j in range(NQB - 1, -1, -1):
            n_i = 4 * j + 4  # causal: k blocks 0 .. 4j+3
            ctx_A = psum_c.tile([D + 1, QB], F32, tag="ctxA", name="ctxA")
            ctx_B = psum_c.tile([D + 1, QB], F32, tag="ctxB", name="ctxB")
            prev = None
            for i in range(n_i):
                t = i - 4 * j
                qs = max(0, 128 * t)  # q start within the 512 block
                s_ps = psum_s.tile([128, 2, QB], F32, tag="s")
                mm1_pair(pr, j, i, s_ps)
                if prev is not None:
                    mm2_pair(pr, j, prev[0], n_i, prev[1], prev[2], ctx_A, ctx_B)
                p_sb = pp.tile([128, 2, QB], BF16, tag="p")
                nc.scalar.activation(
                    p_sb[:, :, qs:QB],
                    s_ps[:, :, qs:QB],
                    mybir.ActivationFunctionType.Exp,
                    scale=1.0 / NORM,
                )
                if t >= 0:
                    # Diagonal sub-block: zero the strict upper triangle
                    # (q < k) of P for both heads, post-exp, on GpSimd.
                    nc.gpsimd.affine_select(
                        out=p_sb[:, :, qs : qs + 128],
                        in_=p_sb[:, :, qs : qs + 128],
                        compare_op=mybir.AluOpType.is_ge,
                        fill=0.0,
                        base=0,
                        pattern=[[0, 2], [1, 128]],  # iota over q, ignore head
                        channel_multiplier=-1,  # -k per partition
                    )
                prev = (i, p_sb, qs)
            mm2_pair(pr, j, prev[0], n_i, prev[1], prev[2], ctx_A, ctx_B)
            # Epilogue: copy unnormalized ctx^T + denom row out via DVE.
            for g, ctx in ((2 * pr, ctx_A), (2 * pr + 1, ctx_B)):
                ctxn = ep.tile([D + 1, QB], F32, tag="ctxn")
                nc.vector.tensor_copy(ctxn, ctx)
                nc.sync.dma_start(
                    out=out[g, :, j * QB : (j + 1) * QB], in_=ctxn
                )


def _build_nc():
    nc = bacc.Bacc(
        "TRN2", target_bir_lowering=False, debug=False, num_devices=NCORES
    )
    with tile.TileContext(nc) as tc, ExitStack() as ctx_stack:
        build_attention(nc, tc, ctx_stack)
    nc.compile()
    return nc


_NC_CACHE = {}


def get_nc():
    if "nc" not in _NC_CACHE:
        _NC_CACHE["nc"] = _build_nc()
    return _NC_CACHE["nc"]


def shard_inputs(query_layer, key_layer, value_layer):
    """Full [SQ, B, H, D] fp32 inputs -> list of 8 per-core input dicts."""
    import ml_dtypes

    bf16 = np.dtype(ml_dtypes.bfloat16)
    q = np.asarray(query_layer, dtype=np.float32)
    k = np.asarray(key_layer, dtype=np.float32)
    v = np.asarray(value_layer, dtype=np.float32)
    # [SQ, B, H, D] -> [B*H, D, SQ] (d-major) for Q/K; [B*H, SQ, D] for V.
    qt = np.ascontiguousarray(q.transpose(1, 2, 3, 0).reshape(B * H, D, SQ))
    kt = np.ascontiguousarray(k.transpose(1, 2, 3, 0).reshape(B * H, D, SQ))
    vn = v.transpose(1, 2, 0, 3).reshape(B * H, SQ, D)
    vn = np.concatenate([vn, np.ones((B * H, SQ, 1), np.float32)], axis=2)
    qt = qt.astype(bf16)
    kt = kt.astype(bf16)
    vn = np.ascontiguousarray(vn.astype(bf16))
    in_maps = []
    for c in range(NCORES):
        sl = slice(HPC * c, HPC * (c + 1))
        in_maps.append(
            {
                "qt": np.ascontiguousarray(qt[sl].reshape(HPC * D, SQ)),
                "kt": np.ascontiguousarray(kt[sl].reshape(HPC * D, SQ)),
                "v": np.ascontiguousarray(vn[sl]),
            }
        )
    return in_maps


def gather_outputs(results):
    """8 per-core {'out': [4, 65, 2048]} -> full [SQ, B, H*D] fp32."""
    raw = np.stack([np.asarray(results[c]["out"], dtype=np.float32) for c in range(NCORES)])
    raw = raw.reshape(B * H, D + 1, SQ)
    ctx_t = raw[:, :D, :] / raw[:, D : D + 1, :]  # normalize by denominator
    full = ctx_t.transpose(2, 0, 1).reshape(SQ, B, H * D)
    return np.ascontiguousarray(full.astype(np.float32))


def run_on_device(in_maps, trace=False):
    from concourse.bass_utils import run_bass_kernel_spmd

    nc = get_nc()
    res = run_bass_kernel_spmd(
        nc, in_maps, core_ids=list(range(NCORES)), trace=trace
    )
    return res


def kernel(query_layer, key_layer, value_layer, attention_mask=None):
    in_maps = shard_inputs(query_layer, key_layer, value_layer)
    res = run_on_device(in_maps, trace=False)
    return gather_outputs(res.results)


# revision 9
# speedup vs baseline: 2.1402x; 1.0197x over previous
"""Trainium2 Bass kernel for nn_CoreAttention (causal attention).

Problem (hardcoded): Q/K/V [SQ=2048, B=2, H=16, D=64] fp32, causal mask,
softmax(QK^T/8) @ V, output [2048, 2, 1024].

Sharding: batch*heads (32) split 4 heads per core across 8 cores.

Per-core device layout (host prepares these in the shard step):
  qt  [256, 2048] bf16 : Q^T d-major; row = pair*128 + head_local*64 + d
  kt  [256, 2048] bf16 : K^T same layout
  v   [4, 2048, 65]    : V natural per head + ones column (denominator)
  out [4, 65, 2048] f32: rows 0-63 unnormalized context^T, row 64 the
                         softmax denominator; host divides + transposes.

Algorithm per head-pair (2 heads packed on 128 SBUF partitions):
  For each q-block j (512 wide), software-pipelined over k-blocks i
  (128 wide, causally trimmed):
    BMM1  S^T[k, q] for both heads via PE row-tiled pair -> PSUM [128,2,512]
    exp   ScalarE activation (scale=1/8) PSUM -> bf16 P in SBUF
    mask  diagonal 128x128 sub-block zeroed on GpSimd (affine_select)
          after exp (keeps mask off the BMM1->exp critical path)
    BMM2  ctx^T[65, q] += V'.T @ P^T per head (V' has ones column; row 64
          accumulates the denominator), issued one i behind BMM1 so the
          PE never waits on the ACT engine.
  Epilogue: DVE copy ctx PSUM->SBUF, DMA out (no on-device normalize).
"""

import sys

sys.path.insert(0, "/opt/trn_rl_repo")

import numpy as np

from contextlib import ExitStack

import concourse.bass as bass
import concourse.mybir as mybir
import concourse.tile as tile
from concourse import bacc

SQ, B, H, D = 2048, 2, 16, 64
NCORES = 8
HPC = 4  # heads per core
NPAIR = 2  # head pairs per core
KB = 128  # k block
QB = 512  # q block
NKB = SQ // KB  # 16
NQB = SQ // QB  # 4
NORM = 8.0  # sqrt(D) * layer_number

F32 = mybir.dt.float32
BF16 = mybir.dt.bfloat16


def build_attention(nc, tc, ctx_stack):
    qt = nc.dram_tensor("qt", [NPAIR * 128, SQ], BF16, kind="ExternalInput").ap()
    kt = nc.dram_tensor("kt", [NPAIR * 128, SQ], BF16, kind="ExternalInput").ap()
    # v carries a host-prepared ones column at d=D (softmax denominator
    # trick) and is pre-swizzled to the SBUF layout [128, HPC, NKB, 65].
    v = nc.dram_tensor(
        "v", [128, HPC * NKB * (D + 1)], BF16, kind="ExternalInput"
    ).ap()
    out = nc.dram_tensor("out", [HPC, D + 1, SQ], F32, kind="ExternalOutput").ap()

    ec = ctx_stack.enter_context
    consts = ec(tc.tile_pool(name="consts", bufs=1))
    inp = ec(tc.tile_pool(name="inp", bufs=1))
    pp = ec(tc.tile_pool(name="pp", bufs=3))
    ep = ec(tc.tile_pool(name="ep", bufs=4))
    psum_s = ec(tc.tile_pool(name="psum_s", bufs=2, space="PSUM"))
    psum_c = ec(tc.tile_pool(name="psum_c", bufs=2, space="PSUM"))

    # 0/1 bf16 causal mask for the diagonal 128x128 sub-block in P^T
    # layout (partition = k, free = q): 1 where q >= k else 0.
    mask_sb = consts.tile([128, 128], BF16)
    nc.gpsimd.memset(mask_sb, 1.0)
    nc.gpsimd.affine_select(
        out=mask_sb,
        in_=mask_sb,
        compare_op=mybir.AluOpType.is_ge,
        fill=0.0,
        base=0,
        pattern=[[1, 128]],  # iota over free dim: +q
        channel_multiplier=-1,  # -k per partition
    )

    # Resident inputs.
    qt_sb = inp.tile([128, NPAIR, SQ], BF16)
    kt_sb = inp.tile([128, NPAIR, SQ], BF16)
    vp_sb = inp.tile([128, HPC, NKB, D + 1], BF16)

    # Chunked input loads, ordered by first use (j runs descending, k
    # ascending): kt chunks ascending, qt chunks descending, vp ascending.
    # v arrives host-pre-swizzled to the SBUF layout so each DMA chunk is
    # partition-linear.
    qt_r = qt.rearrange("(pr p) q -> p pr q", p=128)
    kt_r = kt.rearrange("(pr p) q -> p pr q", p=128)
    v_r = v.rearrange("p (g n d) -> p g n d", g=HPC, n=NKB)
    for c in range(NQB):
        ksl = slice(c * QB, (c + 1) * QB)
        qsl = slice((NQB - 1 - c) * QB, (NQB - c) * QB)
        for pr in range(NPAIR):
            nc.sync.dma_start(out=kt_sb[:, pr, ksl], in_=kt_r[:, pr, ksl])
        for pr in range(NPAIR):
            nc.sync.dma_start(out=qt_sb[:, pr, qsl], in_=qt_r[:, pr, qsl])
        bl = slice(4 * c, 4 * c + 4)
        for g in range(HPC):
            nc.sync.dma_start(out=vp_sb[:, g, bl, :], in_=v_r[:, g, bl, :])

    def mm1_pair(pr, j, i, s_ps):
        """BMM1: S^T[k, q] for both heads, row-tiled on the PE."""
        t = i - 4 * j
        qs = max(0, 128 * t)
        nc.tensor.matmul(
            s_ps[:, 0, qs:QB],
            lhsT=kt_sb[0:64, pr, i * KB : (i + 1) * KB],
            rhs=qt_sb[0:64, pr, j * QB + qs : (j + 1) * QB],
            start=True,
            stop=True,
            tile_position=(0, 0),
        )
        nc.tensor.matmul(
            s_ps[:, 1, qs:QB],
            lhsT=kt_sb[64:128, pr, i * KB : (i + 1) * KB],
            rhs=qt_sb[64:128, pr, j * QB + qs : (j + 1) * QB],
            start=True,
            stop=True,
            tile_position=(64, 0),
        )

    def mm2_pair(pr, j, i, n_i, p_sb, qs, ctx_A, ctx_B):
        """BMM2: ctx^T[0:64] += V.T @ P^T ; row 64 accumulates sums."""
        nc.tensor.matmul(
            ctx_A[:, qs:QB],
            lhsT=vp_sb[:, 2 * pr, i, :],
            rhs=p_sb[:, 0, qs:QB],
            start=(i == 0),
            stop=(i == n_i - 1),
        )
        nc.tensor.matmul(
            ctx_B[:, qs:QB],
            lhsT=vp_sb[:, 2 * pr + 1, i, :],
            rhs=p_sb[:, 1, qs:QB],
            start=(i == 0),
            stop=(i == n_i - 1),
        )

    # Flat slot list across all (pair, j, i) so the software pipeline runs
    # through j/pair boundaries without a bubble. j descending (longest
    # i-loops first, so the kernel tail is short).
    slots = []
    for pr in range(NPAIR):
        for j in range(NQB - 1, -1, -1):
            n_i = 4 * j + 4  # causal: k blocks 0 .. 4j+3
            for i in range(n_i):
                slots.append((pr, j, i, n_i))

    prev = None  # (pr, j, i, n_i, p_sb, qs, ctx_A, ctx_B)
    ctxs = {}
    for pr, j, i, n_i in slots:
        if i == 0:
            ctxs[(pr, j)] = (
                psum_c.tile([D + 1, QB], F32, tag="ctxA", name="ctxA"),
                psum_c.tile([D + 1, QB], F32, tag="ctxB", name="ctxB"),
            )
        ctx_A, ctx_B = ctxs[(pr, j)]
        t = i - 4 * j
        qs = max(0, 128 * t)  # q start within the 512 block
        s_ps = psum_s.tile([128, 2, QB], F32, tag="s")
        mm1_pair(pr, j, i, s_ps)
        if prev is not None:
            mm2_pair(*prev)
            if prev[2] == prev[3] - 1:  # finished a (pair, j): epilogue
                ppr, pj = prev[0], prev[1]
                for g, ctx in ((2 * ppr, prev[6]), (2 * ppr + 1, prev[7])):
                    ctxn = ep.tile([D + 1, QB], F32, tag="ctxn")
                    nc.vector.tensor_copy(ctxn, ctx)
                    nc.sync.dma_start(
                        out=out[g, :, pj * QB : (pj + 1) * QB], in_=ctxn
                    )
        p_sb = pp.tile([128, 2, QB], BF16, tag="p")
        nc.scalar.activation(
            p_sb[:, :, qs:QB],
            s_ps[:, :, qs:QB],
            mybir.ActivationFunctionType.Exp,
            scale=1.0 / NORM,
        )
        if t >= 0:
            # Diagonal sub-block: zero the strict upper triangle (q < k)
            # of P for both heads, post-exp, on DVE (0/1 mask multiply).
            nc.vector.tensor_mul(
                p_sb[:, :, qs : qs + 128],
                p_sb[:, :, qs : qs + 128],
                mask_sb.unsqueeze(1).broadcast_to((128, 2, 128)),
            )
        prev = (pr, j, i, n_i, p_sb, qs, ctx_A, ctx_B)
    mm2_pair(*prev)
    ppr, pj = prev[0], prev[1]
    for g, ctx in ((2 * ppr, prev[6]), (2 * ppr + 1, prev[7])):
        ctxn = ep.tile([D + 1, QB], F32, tag="ctxn")
        nc.vector.tensor_copy(ctxn, ctx)
        nc.sync.dma_start(out=out[g, :, pj * QB : (pj + 1) * QB], in_=ctxn)


def _build_nc():
    nc = bacc.Bacc(
        "TRN2", target_bir_lowering=False, debug=False, num_devices=NCORES
    )
    with tile.TileContext(nc) as tc, ExitStack() as ctx_stack:
        build_attention(nc, tc, ctx_stack)
    nc.compile()
    return nc


_NC_CACHE = {}


def get_nc():
    if "nc" not in _NC_CACHE:
        _NC_CACHE["nc"] = _build_nc()
    return _NC_CACHE["nc"]


def shard_inputs(query_layer, key_layer, value_layer):
    """Full [SQ, B, H, D] fp32 inputs -> list of 8 per-core input dicts."""
    import ml_dtypes

    bf16 = np.dtype(ml_dtypes.bfloat16)
    q = np.asarray(query_layer, dtype=np.float32)
    k = np.asarray(key_layer, dtype=np.float32)
    v = np.asarray(value_layer, dtype=np.float32)
    # [SQ, B, H, D] -> [B*H, D, SQ] (d-major) for Q/K; [B*H, SQ, D] for V.
    qt = np.ascontiguousarray(q.transpose(1, 2, 3, 0).reshape(B * H, D, SQ))
    kt = np.ascontiguousarray(k.transpose(1, 2, 3, 0).reshape(B * H, D, SQ))
    vn = v.transpose(1, 2, 0, 3).reshape(B * H, SQ, D)
    vn = np.concatenate([vn, np.ones((B * H, SQ, 1), np.float32)], axis=2)
    qt = qt.astype(bf16)
    kt = kt.astype(bf16)
    vn = np.ascontiguousarray(vn.astype(bf16))
    in_maps = []
    for c in range(NCORES):
        sl = slice(HPC * c, HPC * (c + 1))
        # Swizzle v to the device SBUF layout [128, HPC, NKB, 65].
        vc = vn[sl].reshape(HPC, NKB, 128, D + 1).transpose(2, 0, 1, 3)
        in_maps.append(
            {
                "qt": np.ascontiguousarray(qt[sl].reshape(HPC * D, SQ)),
                "kt": np.ascontiguousarray(kt[sl].reshape(HPC * D, SQ)),
                "v": np.ascontiguousarray(vc.reshape(128, HPC * NKB * (D + 1))),
            }
        )
    return in_maps


def gather_outputs(results):
    """8 per-core {'out': [4, 65, 2048]} -> full [SQ, B, H*D] fp32."""
    raw = np.stack([np.asarray(results[c]["out"], dtype=np.float32) for c in range(NCORES)])
    raw = raw.reshape(B * H, D + 1, SQ)
    ctx_t = raw[:, :D, :] / raw[:, D : D + 1, :]  # normalize by denominator
    full = ctx_t.transpose(2, 0, 1).reshape(SQ, B, H * D)
    return np.ascontiguousarray(full.astype(np.float32))


def run_on_device(in_maps, trace=False):
    from concourse.bass_utils import run_bass_kernel_spmd

    nc = get_nc()
    res = run_bass_kernel_spmd(
        nc, in_maps, core_ids=list(range(NCORES)), trace=trace
    )
    return res


def kernel(query_layer, key_layer, value_layer, attention_mask=None):
    in_maps = shard_inputs(query_layer, key_layer, value_layer)
    res = run_on_device(in_maps, trace=False)
    return gather_outputs(res.results)
